# revision 7
# baseline (speedup 1.0000x reference)
"""Multi-head attention (B=4, S=2048, D=768, H=12, d=64) on 8 Trainium2 cores.

Sharding: core (b, g) = batch b in [0,4), head-group g in [0,2) — 6 heads each.
Each core computes the qkv projection for its heads, attention, and a partial
(transposed) output projection; the host sums the two head-group partials per
batch and adds b_proj.

Dataflow (per core, all matmuls in float32r = E8M11, full PE rate):
  - host passes x[b].T so contraction dims land on SBUF partitions
  - qkT[768,2048] = wqk.T @ xT   (q rows pre-scaled by 1/8 on host)
  - V[2048,768] = xT.T @ wv + bv, per-head layout [v_h | ones(64)]
  - per head pair, per 1024-col i-half, per 128-row j-tile:
      ST[j,i] = kT.T @ qT   (K=64 contraction; head parity alternates base
      partition 0/64 — alternating row-groups keeps the PE at full rate)
      pT = exp(ST)          (ACT, PSUM->SBUF, float32r out)
      av += [v_h | ones].T @ pT   (rows 0-63: out^T, rows 64-127: softmax
      denominator replicated — the ones-columns trick)
    aoT = av[0:64] * reciprocal(av[64:128])
  - outT[768,2048] = wp.T @ aoT  (partial; host: out[b] = outT_g0.T + outT_g1.T + b_proj)
"""
import numpy as np

B, S, D = 4, 2048, 768
H, DH = 12, 64
HPC = 6          # heads per core
NKT = D // 128   # 6 contraction tiles of 128
NSC = S // 512   # 4 column chunks of 512
NST = S // 128   # 16 row tiles of 128
NJT = 6          # qk projection output row tiles (768/128)
VW = HPC * 128  # v tile width: per head [v_h (64) | ones (64)]

_NC_CACHE = {}


def _round_fp32r(x):
    """Round fp32 to the fp32r grid (E8M11: low 12 mantissa bits zero, RNE)."""
    x = np.ascontiguousarray(x, dtype=np.float32)
    u = x.view(np.uint32).astype(np.uint64)
    u = (u + 0x7FF + ((u >> 12) & 1)) & 0xFFFFF000
    return u.astype(np.uint32).view(np.float32)


def _build_nc():
    import concourse.bass as bass
    import concourse.mybir as mybir
    import concourse.tile as tile
    from concourse import bacc

    f32r = mybir.dt.float32r
    f32 = mybir.dt.float32
    Exp = mybir.ActivationFunctionType.Exp

    nc = bacc.Bacc("TRN2", target_bir_lowering=False, debug=False)
    xT = nc.dram_tensor("xT", [D, S], f32r, kind="ExternalInput").ap()
    wqk = nc.dram_tensor("wqk", [D, 768], f32r, kind="ExternalInput").ap()
    bqk = nc.dram_tensor("bqk", [128, NJT], f32, kind="ExternalInput").ap()
    wv = nc.dram_tensor("wv", [D, 384], f32r, kind="ExternalInput").ap()
    bv = nc.dram_tensor("bv", [128, 384], f32, kind="ExternalInput").ap()
    wp = nc.dram_tensor("wp", [384, D], f32r, kind="ExternalInput").ap()
    outT = nc.dram_tensor("outT", [D, S], f32, kind="ExternalOutput").ap()

    with tile.TileContext(nc) as tc:
        with (
            tc.tile_pool(name="persist", bufs=1) as pp,
            tc.tile_pool(name="stage", bufs=4) as stg,
            tc.tile_pool(name="rec", bufs=2) as recp,
        ):
            # ---- persistent SBUF tensors ----
            qT_t = [pp.tile([128, S], f32r, name=f"qT{i}") for i in range(3)]
            kT_t = [pp.tile([128, S], f32r, name=f"kT{i}") for i in range(3)]
            v_t = [pp.tile([128, VW], f32r, name=f"v{i}") for i in range(NST)]
            bqk_t = pp.tile([128, NJT], f32, name="bqk")
            bv_t = pp.tile([128, 384], f32, name="bv")

            nc.sync.dma_start(bqk_t[:], bqk)
            nc.sync.dma_start(bv_t[:], bv)
            def v_strided(st, off):
                base = v_t[st][:]
                return bass.AP(
                    base.tensor, base.offset + off,
                    [base.ap[0], [128, HPC], [1, 64]],
                )


            with (
                tc.tile_pool(name="xt", bufs=1) as xtp,
                tc.tile_pool(name="w", bufs=1) as wps,
                tc.tile_pool(name="psA", bufs=4, space="PSUM") as psA,
            ):
                xt_t = [
                    [xtp.tile([128, 512], f32r, name=f"xt{k}_{s}") for s in range(NSC)]
                    for k in range(NKT)
                ]
                wqk_t = [wps.tile([128, 768], f32r, name=f"wqk{k}") for k in range(NKT)]
                wv_t = [wps.tile([128, 384], f32r, name=f"wv{k}") for k in range(NKT)]
                for k in range(NKT):
                    nc.sync.dma_start(wqk_t[k][:], wqk[k * 128:(k + 1) * 128, :])
                    nc.sync.dma_start(wv_t[k][:], wv[k * 128:(k + 1) * 128, :])
                    for s in range(NSC):
                        nc.sync.dma_start(
                            xt_t[k][s][:], xT[k * 128:(k + 1) * 128, s * 512:(s + 1) * 512]
                        )

                # ---- phase A: qkT[768, 2048] = wqk.T @ xT (+bias) ----
                for jt in range(NJT):
                    for sc in range(NSC):
                        ps = psA.tile([128, 512], f32, tag="a", name="psa")
                        for kt in range(NKT):
                            nc.tensor.matmul(
                                ps[:],
                                wqk_t[kt][:, jt * 128:(jt + 1) * 128],
                                xt_t[kt][sc][:],
                                start=(kt == 0), stop=(kt == NKT - 1),
                            )
                        csl = slice(sc * 512, (sc + 1) * 512)
                        if jt < 3:
                            nc.vector.tensor_scalar_add(
                                qT_t[jt][:, csl], ps[:], bqk_t[:, jt:jt + 1]
                            )
                        else:
                            nc.vector.tensor_scalar_add(
                                kT_t[jt - 3][:, csl], ps[:], bqk_t[:, jt:jt + 1]
                            )

                # ---- phase B: V[2048, 384] = xT.T @ wv (+bias) ----
                for st in range(NST):
                    ps = psA.tile([128, 512], f32, tag="a", name="psb")
                    for kt in range(NKT):
                        nc.tensor.matmul(
                            ps[:, 0:384],
                            xt_t[kt][st // 4][:, (st % 4) * 128:(st % 4 + 1) * 128],
                            wv_t[kt][:],
                            start=(kt == 0), stop=(kt == NKT - 1),
                        )
                    ps384 = bass.AP(ps.tensor, ps.offset, [ps.ap[0], [64, HPC], [1, 64]])
                    bvb = bv_t[:]
                    bv384 = bass.AP(bvb.tensor, bvb.offset, [bvb.ap[0], [64, HPC], [1, 64]])
                    nc.vector.tensor_add(v_strided(st, 0), ps384, bv384)
                    # ones columns: bv*0 + 1 (finite source, f32r-rounded output)
                    nc.vector.tensor_scalar(
                        v_strided(st, 64), bv384, 0.0, 1.0,
                        mybir.AluOpType.mult, mybir.AluOpType.add,
                    )

            # ---- phases C+D share aoT / wp ----
            with tc.tile_pool(name="late", bufs=1) as lp:
                aoT_t = [lp.tile([128, S], f32r, name=f"aoT{i}") for i in range(3)]
                wp_t = [lp.tile([128, D], f32r, name=f"wp{i}") for i in range(3)]
                for i in range(3):
                    nc.sync.dma_start(wp_t[i][:], wp[i * 128:(i + 1) * 128, :])

                # ---- phase C: attention, head-pair parity interleaved ----
                with (
                    tc.tile_pool(name="pt", bufs=6) as ptp,
                    tc.tile_pool(name="psST", bufs=1, space="PSUM") as psST,
                    tc.tile_pool(name="psAV", bufs=1, space="PSUM") as psAV,
                ):

                    for hp in range(3):
                        qt = qT_t[hp]
                        kt3 = kT_t[hp]
                        for ihalf in range(2):
                            i0 = ihalf * 1024
                            avs = {
                                (par, ic): psAV.tile(
                                    [128, 512], f32,
                                    tag=f"av{par}{ic}", name=f"av{par}{ic}",
                                )
                                for par in range(2) for ic in range(2)
                            }
                            for jt in range(NST):
                                jsl = slice(jt * 128, (jt + 1) * 128)
                                sts = {}
                                for par in range(2):
                                    sts[par] = psST.tile(
                                        [128, 1024], f32,
                                        tag=f"st{par}", name=f"st{par}",
                                    )
                                # strict parity alternation for the K=64 matmuls
                                for ic in range(2):
                                    for par in range(2):
                                        psl = slice(par * 64, par * 64 + 64)
                                        nc.tensor.matmul(
                                            sts[par][:, ic * 512:(ic + 1) * 512],
                                            kt3[psl, jsl],
                                            qt[psl, i0 + ic * 512:i0 + (ic + 1) * 512],
                                            start=True, stop=True,
                                        )
                                pts = {}
                                for par in range(2):
                                    pt = ptp.tile([128, 1024], f32r, tag="pt", name="pt")
                                    nc.scalar.activation(pt[:], sts[par][:], Exp)
                                    pts[par] = pt
                                for par in range(2):
                                    h = 2 * hp + par
                                    for ic in range(2):
                                        nc.tensor.matmul(
                                            avs[par, ic][:],
                                            v_t[jt][:, h * 128:(h + 1) * 128],
                                            pts[par][:, ic * 512:(ic + 1) * 512],
                                            start=(jt == 0), stop=(jt == NST - 1),
                                        )
                            for par in range(2):
                                for ic in range(2):
                                    av = avs[par, ic]
                                    rec = recp.tile([128, 512], f32, tag="rec", name="rec")
                                    nc.vector.reciprocal(rec[64:128, :], av[64:128, :])
                                    dst = aoT_t[hp][
                                        par * 64:par * 64 + 64,
                                        i0 + ic * 512:i0 + (ic + 1) * 512,
                                    ]
                                    nc.vector.tensor_mul(dst, av[0:64, :], rec[64:128, :])

                # ---- phase D: outT[768, 2048] = wp.T @ aoT ----
                with tc.tile_pool(name="psO", bufs=8, space="PSUM") as psO:
                    for jt2 in range(NJT):
                        for ic in range(NSC):
                            ps = psO.tile([128, 512], f32, tag="o", name="pso")
                            for kt3 in range(3):
                                nc.tensor.matmul(
                                    ps[:],
                                    wp_t[kt3][:, jt2 * 128:(jt2 + 1) * 128],
                                    aoT_t[kt3][:, ic * 512:(ic + 1) * 512],
                                    start=(kt3 == 0), stop=(kt3 == 2),
                                )
                            o = stg.tile([128, 512], f32, tag="os", name="os")
                            nc.vector.tensor_copy(o[:], ps[:])
                            nc.sync.dma_start(
                                outT[jt2 * 128:(jt2 + 1) * 128, ic * 512:(ic + 1) * 512],
                                o[:],
                            )

    nc.compile()
    return nc


def _prep_core_inputs(x, w_qkv, b_qkv, w_proj, b, g):
    q0 = g * HPC * DH            # start col of this group's q block
    qs = slice(q0, q0 + 384)
    ks = slice(768 + q0, 768 + q0 + 384)
    vs = slice(1536 + q0, 1536 + q0 + 384)

    xTc = _round_fp32r(x[b].T)
    wqk_h = np.concatenate([w_qkv[:, qs] * 0.125, w_qkv[:, ks]], axis=1)
    wqk_h = _round_fp32r(wqk_h)
    bqk_flat = np.concatenate([b_qkv[qs] * 0.125, b_qkv[ks]])
    bqk_h = np.ascontiguousarray(bqk_flat.reshape(NJT, 128).T, dtype=np.float32)
    wv_h = _round_fp32r(w_qkv[:, vs])
    bv_h = np.ascontiguousarray(
        np.broadcast_to(b_qkv[vs].astype(np.float32), (128, 384))
    )
    wp_h = _round_fp32r(w_proj[g * 384:(g + 1) * 384, :])
    return {"xT": xTc, "wqk": wqk_h, "bqk": bqk_h, "wv": wv_h, "bv": bv_h, "wp": wp_h}


def kernel(x, w_qkv, b_qkv, w_proj, b_proj):
    from concourse.bass_utils import run_bass_kernel_spmd

    x = np.asarray(x, dtype=np.float32)
    w_qkv = np.asarray(w_qkv, dtype=np.float32)
    b_qkv = np.asarray(b_qkv, dtype=np.float32)
    w_proj = np.asarray(w_proj, dtype=np.float32)
    b_proj = np.asarray(b_proj, dtype=np.float32)

    if "nc" not in _NC_CACHE:
        _NC_CACHE["nc"] = _build_nc()
    nc = _NC_CACHE["nc"]

    in_maps = [
        _prep_core_inputs(x, w_qkv, b_qkv, w_proj, core // 2, core % 2)
        for core in range(8)
    ]
    res = run_bass_kernel_spmd(nc, in_maps, core_ids=list(range(8)))

    out = np.empty((B, S, D), dtype=np.float32)
    for b in range(B):
        t0 = res.results[2 * b]["outT"]
        t1 = res.results[2 * b + 1]["outT"]
        out[b] = (t0.T + t1.T) + b_proj
    return out


# revision 8
# speedup vs baseline: 23.6167x; 23.6167x over previous
"""Multi-head attention (B=4, S=2048, D=768, H=12, d=64) on 8 Trainium2 cores.

Sharding: core (b, g) = batch b in [0,4), head-group g in [0,2) — 6 heads each.
Each core computes the qkv projection for its heads, attention, and a partial
(transposed) output projection; the host sums the two head-group partials per
batch and adds b_proj.

Dataflow (per core, all matmuls in float32r = E8M11, full PE rate):
  - host passes x[b].T so contraction dims land on SBUF partitions
  - qkT[768,2048] = wqk.T @ xT   (q rows pre-scaled by 1/8 on host)
  - V[2048,768] = xT.T @ wv + bv, per-head layout [v_h | ones(64)]
  - per head pair, per 1024-col i-half, per 128-row j-tile:
      ST[j,i] = kT.T @ qT   (K=64 contraction; head parity alternates base
      partition 0/64 — alternating row-groups keeps the PE at full rate)
      pT = exp(ST)          (ACT, PSUM->SBUF, float32r out)
      av += [v_h | ones].T @ pT   (rows 0-63: out^T, rows 64-127: softmax
      denominator replicated — the ones-columns trick)
    aoT = av[0:64] * reciprocal(av[64:128])
  - outT[768,2048] = wp.T @ aoT  (partial; host: out[b] = outT_g0.T + outT_g1.T + b_proj)
"""
import numpy as np

B, S, D = 4, 2048, 768
H, DH = 12, 64
HPC = 6          # heads per core
NKT = D // 128   # 6 contraction tiles of 128
NSC = S // 512   # 4 column chunks of 512
NST = S // 128   # 16 row tiles of 128
NJT = 6          # qk projection output row tiles (768/128)
VW = HPC * 128  # v tile width: per head [v_h (64) | ones (64)]

_NC_CACHE = {}


def _round_fp32r(x):
    """Round fp32 to the fp32r grid (E8M11: low 12 mantissa bits zero, RNE)."""
    x = np.ascontiguousarray(x, dtype=np.float32)
    u = x.view(np.uint32).astype(np.uint64)
    u = (u + 0x7FF + ((u >> 12) & 1)) & 0xFFFFF000
    return u.astype(np.uint32).view(np.float32)


def _build_nc():
    import concourse.bass as bass
    import concourse.mybir as mybir
    import concourse.tile as tile
    from concourse import bacc

    f32r = mybir.dt.float32r
    f32 = mybir.dt.float32
    Exp = mybir.ActivationFunctionType.Exp

    nc = bacc.Bacc("TRN2", target_bir_lowering=False, debug=False)
    xT = nc.dram_tensor("xT", [D, S], f32r, kind="ExternalInput").ap()
    wqk = nc.dram_tensor("wqk", [D, 768], f32r, kind="ExternalInput").ap()
    bqk = nc.dram_tensor("bqk", [128, NJT], f32, kind="ExternalInput").ap()
    wv = nc.dram_tensor("wv", [D, 384], f32r, kind="ExternalInput").ap()
    bv = nc.dram_tensor("bv", [128, 384], f32, kind="ExternalInput").ap()
    wp = nc.dram_tensor("wp", [384, D], f32r, kind="ExternalInput").ap()
    outT = nc.dram_tensor("outT", [D, S], f32, kind="ExternalOutput").ap()

    with tile.TileContext(nc) as tc:
        with (
            tc.tile_pool(name="persist", bufs=1) as pp,
            tc.tile_pool(name="stage", bufs=6) as stg,
            tc.tile_pool(name="rec", bufs=2) as recp,
        ):
            # ---- persistent SBUF tensors ----
            qT_t = [pp.tile([128, S], f32r, name=f"qT{i}") for i in range(3)]
            kT_t = [pp.tile([128, S], f32r, name=f"kT{i}") for i in range(3)]
            v_t = [pp.tile([128, VW], f32r, name=f"v{i}") for i in range(NST)]
            bqk_t = pp.tile([128, NJT], f32, name="bqk")
            bv_t = pp.tile([128, 384], f32, name="bv")

            nc.sync.dma_start(bqk_t[:], bqk)
            nc.sync.dma_start(bv_t[:], bv)
            def v_strided(st, off):
                base = v_t[st][:]
                return bass.AP(
                    base.tensor, base.offset + off,
                    [base.ap[0], [128, HPC], [1, 64]],
                )


            with (
                tc.tile_pool(name="xt", bufs=1) as xtp,
                tc.tile_pool(name="w", bufs=1) as wps,
                tc.tile_pool(name="psA", bufs=6, space="PSUM") as psA,
            ):
                xt_t = [
                    [xtp.tile([128, 512], f32r, name=f"xt{k}_{s}") for s in range(NSC)]
                    for k in range(NKT)
                ]
                wqk_t = [wps.tile([128, 768], f32r, name=f"wqk{k}") for k in range(NKT)]
                wv_t = [wps.tile([128, 384], f32r, name=f"wv{k}") for k in range(NKT)]
                for k in range(NKT):
                    nc.sync.dma_start(wqk_t[k][:], wqk[k * 128:(k + 1) * 128, :])
                    nc.sync.dma_start(wv_t[k][:], wv[k * 128:(k + 1) * 128, :])
                    for s in range(NSC):
                        nc.sync.dma_start(
                            xt_t[k][s][:], xT[k * 128:(k + 1) * 128, s * 512:(s + 1) * 512]
                        )

                # ---- phase A: qkT[768, 2048] = wqk.T @ xT (+bias) ----
                for jt in range(NJT):
                    for sc in range(NSC):
                        ps = psA.tile([128, 512], f32, tag="a", name="psa")
                        for kt in range(NKT):
                            nc.tensor.matmul(
                                ps[:],
                                wqk_t[kt][:, jt * 128:(jt + 1) * 128],
                                xt_t[kt][sc][:],
                                start=(kt == 0), stop=(kt == NKT - 1),
                            )
                        csl = slice(sc * 512, (sc + 1) * 512)
                        if jt < 3:
                            nc.vector.tensor_scalar_add(
                                qT_t[jt][:, csl], ps[:], bqk_t[:, jt:jt + 1]
                            )
                        else:
                            nc.vector.tensor_scalar_add(
                                kT_t[jt - 3][:, csl], ps[:], bqk_t[:, jt:jt + 1]
                            )

                # ---- phase B: V[2048, 384] = xT.T @ wv (+bias) ----
                for st in range(NST):
                    ps = psA.tile([128, 512], f32, tag="a", name="psb")
                    for kt in range(NKT):
                        nc.tensor.matmul(
                            ps[:, 0:384],
                            xt_t[kt][st // 4][:, (st % 4) * 128:(st % 4 + 1) * 128],
                            wv_t[kt][:],
                            start=(kt == 0), stop=(kt == NKT - 1),
                        )
                    ps384 = bass.AP(ps.tensor, ps.offset, [ps.ap[0], [64, HPC], [1, 64]])
                    bvb = bv_t[:]
                    bv384 = bass.AP(bvb.tensor, bvb.offset, [bvb.ap[0], [64, HPC], [1, 64]])
                    nc.vector.tensor_add(v_strided(st, 0), ps384, bv384)
                    # ones columns: bv*0 + 1 (finite source, f32r-rounded output)
                    nc.vector.tensor_scalar(
                        v_strided(st, 64), bv384, 0.0, 1.0,
                        mybir.AluOpType.mult, mybir.AluOpType.add,
                    )

            # ---- phases C+D share aoT / wp ----
            with tc.tile_pool(name="late", bufs=1) as lp:
                aoT_t = [lp.tile([128, S], f32r, name=f"aoT{i}") for i in range(3)]
                wp_t = [lp.tile([128, D], f32r, name=f"wp{i}") for i in range(3)]
                for i in range(3):
                    nc.sync.dma_start(wp_t[i][:], wp[i * 128:(i + 1) * 128, :])

                # ---- phase C: attention, head-pair parity interleaved ----
                with (
                    tc.tile_pool(name="pt", bufs=10) as ptp,
                    tc.tile_pool(name="psST", bufs=1, space="PSUM") as psST,
                    tc.tile_pool(name="psAV", bufs=1, space="PSUM") as psAV,
                ):

                    for hp in range(3):
                        qt = qT_t[hp]
                        kt3 = kT_t[hp]
                        for ihalf in range(2):
                            i0 = ihalf * 1024
                            avs = {
                                (par, ic): psAV.tile(
                                    [128, 512], f32,
                                    tag=f"av{par}{ic}", name=f"av{par}{ic}",
                                )
                                for par in range(2) for ic in range(2)
                            }
                            for jt in range(NST):
                                jsl = slice(jt * 128, (jt + 1) * 128)
                                sts = {}
                                for par in range(2):
                                    sts[par] = psST.tile(
                                        [128, 1024], f32,
                                        tag=f"st{par}", name=f"st{par}",
                                    )
                                # strict parity alternation for the K=64 matmuls
                                for ic in range(2):
                                    for par in range(2):
                                        psl = slice(par * 64, par * 64 + 64)
                                        nc.tensor.matmul(
                                            sts[par][:, ic * 512:(ic + 1) * 512],
                                            kt3[psl, jsl],
                                            qt[psl, i0 + ic * 512:i0 + (ic + 1) * 512],
                                            start=True, stop=True,
                                        )
                                pts = {}
                                for par in range(2):
                                    pt = ptp.tile([128, 1024], f32r, tag="pt", name="pt")
                                    nc.scalar.activation(pt[:], sts[par][:], Exp)
                                    pts[par] = pt
                                for par in range(2):
                                    h = 2 * hp + par
                                    for ic in range(2):
                                        nc.tensor.matmul(
                                            avs[par, ic][:],
                                            v_t[jt][:, h * 128:(h + 1) * 128],
                                            pts[par][:, ic * 512:(ic + 1) * 512],
                                            start=(jt == 0), stop=(jt == NST - 1),
                                        )
                            for par in range(2):
                                for ic in range(2):
                                    av = avs[par, ic]
                                    rec = recp.tile([128, 512], f32, tag="rec", name="rec")
                                    nc.vector.reciprocal(rec[64:128, :], av[64:128, :])
                                    dst = aoT_t[hp][
                                        par * 64:par * 64 + 64,
                                        i0 + ic * 512:i0 + (ic + 1) * 512,
                                    ]
                                    nc.vector.tensor_mul(dst, av[0:64, :], rec[64:128, :])

                # ---- phase D: outT[768, 2048] = wp.T @ aoT ----
                with tc.tile_pool(name="psO", bufs=8, space="PSUM") as psO:
                    for jt2 in range(NJT):
                        for ic in range(NSC):
                            ps = psO.tile([128, 512], f32, tag="o", name="pso")
                            for kt3 in range(3):
                                nc.tensor.matmul(
                                    ps[:],
                                    wp_t[kt3][:, jt2 * 128:(jt2 + 1) * 128],
                                    aoT_t[kt3][:, ic * 512:(ic + 1) * 512],
                                    start=(kt3 == 0), stop=(kt3 == 2),
                                )
                            o = stg.tile([128, 512], f32, tag="os", name="os")
                            nc.vector.tensor_copy(o[:], ps[:])
                            nc.sync.dma_start(
                                outT[jt2 * 128:(jt2 + 1) * 128, ic * 512:(ic + 1) * 512],
                                o[:],
                            )

    nc.compile()
    return nc


def _prep_core_inputs(x, w_qkv, b_qkv, w_proj, b, g):
    q0 = g * HPC * DH            # start col of this group's q block
    qs = slice(q0, q0 + 384)
    ks = slice(768 + q0, 768 + q0 + 384)
    vs = slice(1536 + q0, 1536 + q0 + 384)

    xTc = _round_fp32r(x[b].T)
    wqk_h = np.concatenate([w_qkv[:, qs] * 0.125, w_qkv[:, ks]], axis=1)
    wqk_h = _round_fp32r(wqk_h)
    bqk_flat = np.concatenate([b_qkv[qs] * 0.125, b_qkv[ks]])
    bqk_h = np.ascontiguousarray(bqk_flat.reshape(NJT, 128).T, dtype=np.float32)
    wv_h = _round_fp32r(w_qkv[:, vs])
    bv_h = np.ascontiguousarray(
        np.broadcast_to(b_qkv[vs].astype(np.float32), (128, 384))
    )
    wp_h = _round_fp32r(w_proj[g * 384:(g + 1) * 384, :])
    return {"xT": xTc, "wqk": wqk_h, "bqk": bqk_h, "wv": wv_h, "bv": bv_h, "wp": wp_h}


def kernel(x, w_qkv, b_qkv, w_proj, b_proj):
    from concourse.bass_utils import run_bass_kernel_spmd

    x = np.asarray(x, dtype=np.float32)
    w_qkv = np.asarray(w_qkv, dtype=np.float32)
    b_qkv = np.asarray(b_qkv, dtype=np.float32)
    w_proj = np.asarray(w_proj, dtype=np.float32)
    b_proj = np.asarray(b_proj, dtype=np.float32)

    if "nc" not in _NC_CACHE:
        _NC_CACHE["nc"] = _build_nc()
    nc = _NC_CACHE["nc"]

    in_maps = [
        _prep_core_inputs(x, w_qkv, b_qkv, w_proj, core // 2, core % 2)
        for core in range(8)
    ]
    res = run_bass_kernel_spmd(nc, in_maps, core_ids=list(range(8)))

    out = np.empty((B, S, D), dtype=np.float32)
    for b in range(B):
        t0 = res.results[2 * b]["outT"]
        t1 = res.results[2 * b + 1]["outT"]
        out[b] = (t0.T + t1.T) + b_proj
    return out


# revision 9
# speedup vs baseline: 32.0692x; 1.3579x over previous
"""Multi-head attention (B=4, S=2048, D=768, H=12, d=64) on 8 Trainium2 cores.

Sharding: core (b, g) = batch b in [0,4), head-group g in [0,2) — 6 heads each.
Each core computes the qkv projection for its heads, attention, and a partial
(transposed) output projection; the host sums the two head-group partials per
batch and adds b_proj.

Dataflow (per core, all matmuls in float32r = E8M11, full PE rate):
  - host passes x[b].T so contraction dims land on SBUF partitions
  - qkT[768,2048] = wqk.T @ xT   (q rows pre-scaled by 1/8 on host)
  - V[2048,768] = xT.T @ wv + bv, per-head layout [v_h | ones(64)]
  - per head pair, per 1024-col i-half, per 128-row j-tile:
      ST[j,i] = kT.T @ qT   (K=64 contraction; head parity alternates base
      partition 0/64 — alternating row-groups keeps the PE at full rate)
      pT = exp(ST)          (ACT, PSUM->SBUF, float32r out)
      av += [v_h | ones].T @ pT   (rows 0-63: out^T, rows 64-127: softmax
      denominator replicated — the ones-columns trick)
    aoT = av[0:64] * reciprocal(av[64:128])
  - outT[768,2048] = wp.T @ aoT  (partial; host: out[b] = outT_g0.T + outT_g1.T + b_proj)
"""
import numpy as np

B, S, D = 4, 2048, 768
H, DH = 12, 64
HPC = 6          # heads per core
NKT = D // 128   # 6 contraction tiles of 128
NSC = S // 512   # 4 column chunks of 512
NST = S // 128   # 16 row tiles of 128
NJT = 6          # qk projection output row tiles (768/128)
VW = HPC * 128  # v tile width: per head [v_h (64) | ones (64)]

_NC_CACHE = {}


def _round_fp32r(x):
    """Round fp32 to the fp32r grid (E8M11: low 12 mantissa bits zero, RNE)."""
    x = np.ascontiguousarray(x, dtype=np.float32)
    u = x.view(np.uint32).astype(np.uint64)
    u = (u + 0x7FF + ((u >> 12) & 1)) & 0xFFFFF000
    return u.astype(np.uint32).view(np.float32)


def _build_nc():
    import concourse.bass as bass
    import concourse.mybir as mybir
    import concourse.tile as tile
    from concourse import bacc

    f32r = mybir.dt.float32r
    f32 = mybir.dt.float32
    Exp = mybir.ActivationFunctionType.Exp

    nc = bacc.Bacc("TRN2", target_bir_lowering=False, debug=False)
    xT = nc.dram_tensor("xT", [D, S], f32r, kind="ExternalInput").ap()
    wqk = nc.dram_tensor("wqk", [D, 768], f32r, kind="ExternalInput").ap()
    bqk = nc.dram_tensor("bqk", [128, NJT], f32, kind="ExternalInput").ap()
    wv = nc.dram_tensor("wv", [D, 384], f32r, kind="ExternalInput").ap()
    bv = nc.dram_tensor("bv", [128, 384], f32, kind="ExternalInput").ap()
    wp = nc.dram_tensor("wp", [384, D], f32r, kind="ExternalInput").ap()
    outT = nc.dram_tensor("outT", [D, S], f32, kind="ExternalOutput").ap()

    with tile.TileContext(nc) as tc:
        with (
            tc.tile_pool(name="persist", bufs=1) as pp,
            tc.tile_pool(name="stage", bufs=6) as stg,
            tc.tile_pool(name="rec", bufs=4) as recp,
        ):
            # ---- persistent SBUF tensors ----
            qT_t = [pp.tile([128, S], f32r, name=f"qT{i}") for i in range(3)]
            kT_t = [pp.tile([128, S], f32r, name=f"kT{i}") for i in range(3)]
            v_t = [pp.tile([128, VW], f32r, name=f"v{i}") for i in range(NST)]
            bqk_t = pp.tile([128, NJT], f32, name="bqk")
            bv_t = pp.tile([128, 384], f32, name="bv")

            nc.sync.dma_start(bqk_t[:], bqk)
            nc.sync.dma_start(bv_t[:], bv)
            def v_strided(st, off):
                base = v_t[st][:]
                return bass.AP(
                    base.tensor, base.offset + off,
                    [base.ap[0], [128, HPC], [1, 64]],
                )


            with (
                tc.tile_pool(name="xt", bufs=1) as xtp,
                tc.tile_pool(name="w", bufs=1) as wps,
                tc.tile_pool(name="psA", bufs=8, space="PSUM") as psA,
            ):
                xt_t = [
                    [xtp.tile([128, 512], f32r, name=f"xt{k}_{s}") for s in range(NSC)]
                    for k in range(NKT)
                ]
                wqk_t = [wps.tile([128, 768], f32r, name=f"wqk{k}") for k in range(NKT)]
                wv_t = [wps.tile([128, 384], f32r, name=f"wv{k}") for k in range(NKT)]
                for k in range(NKT):
                    nc.sync.dma_start(wqk_t[k][:], wqk[k * 128:(k + 1) * 128, :])
                    nc.sync.dma_start(wv_t[k][:], wv[k * 128:(k + 1) * 128, :])
                    for s in range(NSC):
                        nc.sync.dma_start(
                            xt_t[k][s][:], xT[k * 128:(k + 1) * 128, s * 512:(s + 1) * 512]
                        )

                # ---- phase A: qkT[768, 2048] = wqk.T @ xT (+bias) ----
                for jt in range(NJT):
                    for sc in range(NSC):
                        ps = psA.tile([128, 512], f32, tag="a", name="psa")
                        for kt in range(NKT):
                            nc.tensor.matmul(
                                ps[:],
                                wqk_t[kt][:, jt * 128:(jt + 1) * 128],
                                xt_t[kt][sc][:],
                                start=(kt == 0), stop=(kt == NKT - 1),
                            )
                        csl = slice(sc * 512, (sc + 1) * 512)
                        if jt < 3:
                            nc.vector.tensor_scalar_add(
                                qT_t[jt][:, csl], ps[:], bqk_t[:, jt:jt + 1]
                            )
                        else:
                            nc.vector.tensor_scalar_add(
                                kT_t[jt - 3][:, csl], ps[:], bqk_t[:, jt:jt + 1]
                            )

                # ---- phase B: V[2048, 384] = xT.T @ wv (+bias) ----
                for st in range(NST):
                    ps = psA.tile([128, 512], f32, tag="a", name="psb")
                    for kt in range(NKT):
                        nc.tensor.matmul(
                            ps[:, 0:384],
                            xt_t[kt][st // 4][:, (st % 4) * 128:(st % 4 + 1) * 128],
                            wv_t[kt][:],
                            start=(kt == 0), stop=(kt == NKT - 1),
                        )
                    ps384 = bass.AP(ps.tensor, ps.offset, [ps.ap[0], [64, HPC], [1, 64]])
                    bvb = bv_t[:]
                    bv384 = bass.AP(bvb.tensor, bvb.offset, [bvb.ap[0], [64, HPC], [1, 64]])
                    nc.vector.tensor_add(v_strided(st, 0), ps384, bv384)
                    # ones columns: bv*0 + 1 (finite source, f32r-rounded output)
                    nc.vector.tensor_scalar(
                        v_strided(st, 64), bv384, 0.0, 1.0,
                        mybir.AluOpType.mult, mybir.AluOpType.add,
                    )

            # ---- phases C+D share aoT / wp ----
            with tc.tile_pool(name="late", bufs=1) as lp:
                aoT_t = [lp.tile([128, S], f32r, name=f"aoT{i}") for i in range(3)]
                wp_t = [lp.tile([128, D], f32r, name=f"wp{i}") for i in range(3)]
                for i in range(3):
                    nc.sync.dma_start(wp_t[i][:], wp[i * 128:(i + 1) * 128, :])

                # ---- phase C: attention, head-pair parity interleaved ----
                with (
                    tc.tile_pool(name="pt", bufs=12) as ptp,
                    tc.tile_pool(name="psST", bufs=1, space="PSUM") as psST,
                    tc.tile_pool(name="psAV", bufs=1, space="PSUM") as psAV,
                ):

                    for hp in range(3):
                        qt = qT_t[hp]
                        kt3 = kT_t[hp]
                        for ihalf in range(2):
                            i0 = ihalf * 1024
                            avs = {
                                (par, ic): psAV.tile(
                                    [128, 512], f32,
                                    tag=f"av{par}{ic}", name=f"av{par}{ic}",
                                )
                                for par in range(2) for ic in range(2)
                            }
                            for jt in range(NST):
                                jsl = slice(jt * 128, (jt + 1) * 128)
                                sts = {}
                                for par in range(2):
                                    sts[par] = psST.tile(
                                        [128, 1024], f32,
                                        tag=f"st{par}", name=f"st{par}",
                                    )
                                # strict parity alternation for the K=64 matmuls
                                for ic in range(2):
                                    for par in range(2):
                                        psl = slice(par * 64, par * 64 + 64)
                                        nc.tensor.matmul(
                                            sts[par][:, ic * 512:(ic + 1) * 512],
                                            kt3[psl, jsl],
                                            qt[psl, i0 + ic * 512:i0 + (ic + 1) * 512],
                                            start=True, stop=True,
                                        )
                                pts = {}
                                for par in range(2):
                                    pt = ptp.tile([128, 1024], f32r, tag="pt", name="pt")
                                    nc.scalar.activation(pt[:], sts[par][:], Exp)
                                    pts[par] = pt
                                for par in range(2):
                                    h = 2 * hp + par
                                    for ic in range(2):
                                        nc.tensor.matmul(
                                            avs[par, ic][:],
                                            v_t[jt][:, h * 128:(h + 1) * 128],
                                            pts[par][:, ic * 512:(ic + 1) * 512],
                                            start=(jt == 0), stop=(jt == NST - 1),
                                        )
                            for par in range(2):
                                for ic in range(2):
                                    av = avs[par, ic]
                                    rec = recp.tile([128, 512], f32, tag="rec", name="rec")
                                    nc.vector.reciprocal(rec[64:128, :], av[64:128, :])
                                    dst = aoT_t[hp][
                                        par * 64:par * 64 + 64,
                                        i0 + ic * 512:i0 + (ic + 1) * 512,
                                    ]
                                    nc.vector.tensor_mul(dst, av[0:64, :], rec[64:128, :])

                # ---- phase D: outT[768, 2048] = wp.T @ aoT ----
                with tc.tile_pool(name="psO", bufs=8, space="PSUM") as psO:
                    for jt2 in range(NJT):
                        for ic in range(NSC):
                            ps = psO.tile([128, 512], f32, tag="o", name="pso")
                            for kt3 in range(3):
                                nc.tensor.matmul(
                                    ps[:],
                                    wp_t[kt3][:, jt2 * 128:(jt2 + 1) * 128],
                                    aoT_t[kt3][:, ic * 512:(ic + 1) * 512],
                                    start=(kt3 == 0), stop=(kt3 == 2),
                                )
                            o = stg.tile([128, 512], f32, tag="os", name="os")
                            nc.vector.tensor_copy(o[:], ps[:])
                            nc.sync.dma_start(
                                outT[jt2 * 128:(jt2 + 1) * 128, ic * 512:(ic + 1) * 512],
                                o[:],
                            )

    nc.compile()
    return nc


def _prep_core_inputs(x, w_qkv, b_qkv, w_proj, b, g):
    q0 = g * HPC * DH            # start col of this group's q block
    qs = slice(q0, q0 + 384)
    ks = slice(768 + q0, 768 + q0 + 384)
    vs = slice(1536 + q0, 1536 + q0 + 384)

    xTc = _round_fp32r(x[b].T)
    wqk_h = np.concatenate([w_qkv[:, qs] * 0.125, w_qkv[:, ks]], axis=1)
    wqk_h = _round_fp32r(wqk_h)
    bqk_flat = np.concatenate([b_qkv[qs] * 0.125, b_qkv[ks]])
    bqk_h = np.ascontiguousarray(bqk_flat.reshape(NJT, 128).T, dtype=np.float32)
    wv_h = _round_fp32r(w_qkv[:, vs])
    bv_h = np.ascontiguousarray(
        np.broadcast_to(b_qkv[vs].astype(np.float32), (128, 384))
    )
    wp_h = _round_fp32r(w_proj[g * 384:(g + 1) * 384, :])
    return {"xT": xTc, "wqk": wqk_h, "bqk": bqk_h, "wv": wv_h, "bv": bv_h, "wp": wp_h}


def kernel(x, w_qkv, b_qkv, w_proj, b_proj):
    from concourse.bass_utils import run_bass_kernel_spmd

    x = np.asarray(x, dtype=np.float32)
    w_qkv = np.asarray(w_qkv, dtype=np.float32)
    b_qkv = np.asarray(b_qkv, dtype=np.float32)
    w_proj = np.asarray(w_proj, dtype=np.float32)
    b_proj = np.asarray(b_proj, dtype=np.float32)

    if "nc" not in _NC_CACHE:
        _NC_CACHE["nc"] = _build_nc()
    nc = _NC_CACHE["nc"]

    in_maps = [
        _prep_core_inputs(x, w_qkv, b_qkv, w_proj, core // 2, core % 2)
        for core in range(8)
    ]
    res = run_bass_kernel_spmd(nc, in_maps, core_ids=list(range(8)))

    out = np.empty((B, S, D), dtype=np.float32)
    for b in range(B):
        t0 = res.results[2 * b]["outT"]
        t1 = res.results[2 * b + 1]["outT"]
        out[b] = (t0.T + t1.T) + b_proj
    return out
